# revision 2
# baseline (speedup 1.0000x reference)
"""Multi-head attention (B=4, N=2048, C=1024, H=16, D=64) on 8 TRN2 cores.

Sharding: core c handles batch b = c // 2 and head-group g = c % 2
(8 heads each). Data-parallel over B, tensor-parallel over heads:
qkv column-parallel, output projection row-parallel with the 2-way
partial-sum reduction (+ bias) done on host during unshard.

Per-core device kernel (all matmuls fp32r = 1-pass PE mode):
  phase A (per 512-wide n-tile): qT/kT via transposed projection from
    pre-transposed x, V in natural layout with a ones column per head.
  phase B: causal attention on S^T tiles; K=64 QK^T matmuls pair-packed
    via tile_position; ACT exp reads the 2-bank PSUM pair directly;
    the V-ones column makes the AV matmul accumulate softmax
    denominators in PSUM row 64; normalize = reciprocal +
    gpsimd partition_broadcast + DVE multiply.
  phase C: row-parallel out-projection of the per-head-group context.
"""
import sys

import numpy as np

sys.path.insert(0, "/opt/trn_rl_repo")

import concourse.mybir as mybir
from concourse import bacc
from concourse.bass_utils import run_bass_kernel_spmd
from concourse.tile import TileContext

F32 = mybir.dt.float32
F32R = mybir.dt.float32r

B, N, C = 4, 2048, 1024
H = 16
D = C // H  # 64
SCALE = D ** -0.5
NCORES = 8
HPC = H // 2  # heads per core = 8
PAIRS = 4    # head pairs per core
NT = N // 512  # 4 n-tiles
MC = N // 128  # 16 m-chunks

_CACHE = {}


def build():
    nc = bacc.Bacc(None, target_bir_lowering=False)
    xt = nc.dram_tensor("xt", [C, N], F32R, kind="ExternalInput")
    wqk = nc.dram_tensor("wqk", [C, 1024], F32R, kind="ExternalInput")
    bqk = nc.dram_tensor("bqk", [128, 8], F32, kind="ExternalInput")
    wv = nc.dram_tensor("wv", [C, 512], F32R, kind="ExternalInput")
    bv = nc.dram_tensor("bv", [1, 512], F32, kind="ExternalInput")
    wp = nc.dram_tensor("wp", [512, C], F32R, kind="ExternalInput")
    out = nc.dram_tensor("out", [N, C], F32, kind="ExternalOutput")

    with TileContext(nc) as tc:
        with (
            tc.tile_pool(name="consts", bufs=1) as consts,
            tc.tile_pool(name="wpool", bufs=1) as wpool,
            tc.tile_pool(name="xtp", bufs=2) as xtp,
            tc.tile_pool(name="qkt", bufs=1) as qkt,
            tc.tile_pool(name="vhat", bufs=1) as vhatp,
            tc.tile_pool(name="ptp", bufs=2) as ptp,
            tc.tile_pool(name="ctx", bufs=1) as ctxp,
            tc.tile_pool(name="small", bufs=2) as small,
            tc.tile_pool(name="outp", bufs=2) as outp,
            tc.tile_pool(name="ps_mm", bufs=2, space="PSUM") as ps_mm,
            tc.tile_pool(name="ps_sc", bufs=2, space="PSUM") as ps_sc,
            tc.tile_pool(name="ps_av", bufs=2, space="PSUM") as ps_av,
        ):
            # ---- constants / weights ----
            wqk_sb = wpool.tile([128, 8, 1024], F32R, name="wqk_sb")
            nc.sync.dma_start(wqk_sb[:], wqk.rearrange("(kc p) o -> p kc o", p=128))
            wv_sb = wpool.tile([128, 8, 512], F32R, name="wv_sb")
            nc.sync.dma_start(wv_sb[:], wv.rearrange("(kc p) o -> p kc o", p=128))
            wp_sb = wpool.tile([128, 4, 1024], F32R, name="wp_sb")
            nc.sync.dma_start(wp_sb[:], wp.rearrange("(kc p) o -> p kc o", p=128))
            bqk_sb = consts.tile([128, 8], F32, name="bqk_sb")
            nc.sync.dma_start(bqk_sb[:], bqk[:])
            bv_sb = consts.tile([1, 512], F32, name="bv_sb")
            nc.sync.dma_start(bv_sb[0:1, :], bv[:])
            bv_bc = consts.tile([128, 512], F32, name="bv_bc")
            nc.gpsimd.partition_broadcast(bv_bc[:, :], bv_sb[0:1, :])
            ones_f = consts.tile([128, 1], F32, name="ones_f")
            nc.vector.memset(ones_f[:], 1.0)

            # persistent attention operands
            kt_sb = qkt.tile([128, 4, N], F32R, name="kt_sb")
            vhat = vhatp.tile([128, MC, HPC, D + 1], F32R, name="vhat")
            # ones columns of v-hat (col D of every (mchunk, head) slot)
            nc.vector.tensor_copy(
                vhat[:, :, :, D], ones_f[:].to_broadcast((128, MC, HPC))
            )

            for nt in range(NT):
                # ---------- phase A: projections for this n-tile ----------
                qt_sb = qkt.tile([128, 4, 512], F32R, name="qt_sb", bufs=2)
                for half in range(2):
                    n0 = nt * 512 + half * 256
                    xt_sb = xtp.tile([128, 8, 256], F32R, name="xt_sb")
                    nc.sync.dma_start(
                        xt_sb[:],
                        xt.rearrange("(kc p) n -> p kc n", p=128)[
                            :, :, n0 : n0 + 256
                        ],
                    )
                    # q (oc 0..3 = pairs) and k (oc 4..7) transposed outputs
                    for oc in range(8):
                        ps = ps_mm.tile([128, 512], F32, name="ps_qk", tag="mm")
                        for kc in range(8):
                            nc.tensor.matmul(
                                ps[:, 0:256],
                                wqk_sb[:, kc, oc * 128 : (oc + 1) * 128],
                                xt_sb[:, kc, :],
                                start=(kc == 0),
                                stop=(kc == 7),
                            )
                        if oc < 4:
                            dest = qt_sb[:, oc, half * 256 : half * 256 + 256]
                        else:
                            dest = kt_sb[:, oc - 4, n0 : n0 + 256]
                        nc.vector.tensor_scalar_add(
                            dest, ps[:, 0:256], bqk_sb[:, oc : oc + 1]
                        )
                    # v natural layout, one 128-row chunk at a time
                    for j in range(2):
                        mc = nt * 4 + half * 2 + j
                        ps = ps_mm.tile([128, 512], F32, name="ps_v", tag="mm")
                        for kc in range(8):
                            nc.tensor.matmul(
                                ps[:],
                                xt_sb[:, kc, j * 128 : (j + 1) * 128],
                                wv_sb[:, kc, :],
                                start=(kc == 0),
                                stop=(kc == 7),
                            )
                        nc.vector.tensor_tensor(
                            vhat[:, mc, :, 0:D],
                            ps.rearrange("p (h d) -> p h d", d=D),
                            bv_bc.rearrange("p (h d) -> p h d", d=D),
                            mybir.AluOpType.add,
                        )

                # ---------- phase B: attention for this n-tile ----------
                ctxt = ctxp.tile([128, 4, 512], F32R, name="ctxt")
                nmc = 4 * (nt + 1)
                for pair in range(PAIRS):
                    av0 = ps_av.tile([128, 512], F32, name="ps_av0", tag="av")
                    av1 = ps_av.tile([128, 512], F32, name="ps_av1", tag="av")
                    for mc in range(nmc):
                        sc = ps_sc.tile([128, 2, 512], F32, name="ps_sc", tag="sc")
                        nc.tensor.matmul(
                            sc[:, 0, :],
                            kt_sb[0:64, pair, mc * 128 : (mc + 1) * 128],
                            qt_sb[0:64, pair, :],
                            start=True,
                            stop=True,
                            tile_position=(0, 0),
                        )
                        nc.tensor.matmul(
                            sc[:, 1, :],
                            kt_sb[64:128, pair, mc * 128 : (mc + 1) * 128],
                            qt_sb[64:128, pair, :],
                            start=True,
                            stop=True,
                            tile_position=(64, 0),
                        )
                        pt = ptp.tile([128, 2, 512], F32R, name="pt")
                        nc.scalar.activation(
                            pt[:, :, :], sc[:, :, :],
                            mybir.ActivationFunctionType.Exp,
                        )
                        di = mc - 4 * nt  # >=0 only for diagonal-square chunks
                        if di >= 0:
                            w = 128 * (di + 1)
                            for hh in range(2):
                                nc.gpsimd.affine_select(
                                    out=pt[:, hh, 0:w],
                                    in_=pt[:, hh, 0:w],
                                    compare_op=mybir.AluOpType.is_ge,
                                    fill=0.0,
                                    base=-128 * di,
                                    pattern=[[1, w]],
                                    channel_multiplier=-1,
                                )
                        nc.tensor.matmul(
                            av0[0:65, :],
                            vhat[:, mc, 2 * pair, :],
                            pt[:, 0, :],
                            start=(mc == 0),
                            stop=(mc == nmc - 1),
                        )
                        nc.tensor.matmul(
                            av1[0:65, :],
                            vhat[:, mc, 2 * pair + 1, :],
                            pt[:, 1, :],
                            start=(mc == 0),
                            stop=(mc == nmc - 1),
                        )
                    # normalize: ctx^T[d, n] / denom[n]
                    for hh, av in ((0, av0), (1, av1)):
                        recip = small.tile([1, 512], F32, name="recip")
                        nc.vector.reciprocal(recip[0:1, :], av[64:65, :])
                        bc = small.tile([128, 512], F32, name="bc")
                        nc.gpsimd.partition_broadcast(bc[0:64, :], recip[0:1, :])
                        if hh == 0:
                            nc.vector.tensor_tensor(
                                ctxt[0:64, pair, :], av[0:64, :], bc[0:64, :],
                                mybir.AluOpType.mult,
                            )
                        else:
                            tmp = small.tile([64, 512], F32R, name="tmp")
                            nc.vector.tensor_tensor(
                                tmp[0:64, :], av[0:64, :], bc[0:64, :],
                                mybir.AluOpType.mult,
                            )
                            nc.sync.dma_start(
                                ctxt[64:128, pair, :], tmp[0:64, :]
                            )

                # ---------- phase C: out-projection rows of this n-tile ----
                for j in range(4):
                    for half in range(2):
                        ps = ps_mm.tile([128, 512], F32, name="ps_o", tag="mm")
                        for kc in range(4):
                            nc.tensor.matmul(
                                ps[:],
                                ctxt[:, kc, j * 128 : (j + 1) * 128],
                                wp_sb[:, kc, half * 512 : half * 512 + 512],
                                start=(kc == 0),
                                stop=(kc == 3),
                            )
                        so = outp.tile([128, 512], F32, name="so")
                        nc.vector.tensor_copy(so[:], ps[:])
                        nc.sync.dma_start(
                            out[
                                nt * 512 + j * 128 : nt * 512 + (j + 1) * 128,
                                half * 512 : half * 512 + 512,
                            ],
                            so[:],
                        )
    nc.finalize()
    return nc


def kernel(x, w_qkv, b_qkv, w_proj, b_proj, mask, _collect=None):
    x = np.ascontiguousarray(np.asarray(x, dtype=np.float32))
    w_qkv = np.asarray(w_qkv, dtype=np.float32)
    b_qkv = np.asarray(b_qkv, dtype=np.float32)
    w_proj = np.asarray(w_proj, dtype=np.float32)
    b_proj = np.asarray(b_proj, dtype=np.float32)

    if "nc" not in _CACHE:
        _CACHE["nc"] = build()
    nc = _CACHE["nc"]

    in_maps = []
    for c in range(NCORES):
        b = c // 2
        g = c % 2
        h0 = g * 512  # first of this group's 512 qkv columns per section
        wq = w_qkv[:, h0 : h0 + 512] * np.float32(SCALE)
        wk = w_qkv[:, 1024 + h0 : 1024 + h0 + 512]
        wv_ = w_qkv[:, 2048 + h0 : 2048 + h0 + 512]
        bq = b_qkv[h0 : h0 + 512] * np.float32(SCALE)
        bk = b_qkv[1024 + h0 : 1024 + h0 + 512]
        bv_ = b_qkv[2048 + h0 : 2048 + h0 + 512]
        bqk = np.concatenate([bq, bk]).reshape(8, 128).T  # [128, 8]
        in_maps.append(
            {
                "xt": np.ascontiguousarray(x[b].T),
                "wqk": np.ascontiguousarray(np.concatenate([wq, wk], axis=1)),
                "bqk": np.ascontiguousarray(bqk),
                "wv": np.ascontiguousarray(wv_),
                "bv": np.ascontiguousarray(bv_.reshape(1, 512)),
                "wp": np.ascontiguousarray(w_proj[h0 : h0 + 512, :]),
            }
        )

    trace = _collect is not None and _collect.get("trace", False)
    r = run_bass_kernel_spmd(
        nc, in_maps, core_ids=list(range(NCORES)), trace=trace
    )
    if _collect is not None:
        _collect["results"] = r

    out = np.empty((B, N, C), dtype=np.float32)
    for b in range(B):
        out[b] = r.results[2 * b]["out"] + r.results[2 * b + 1]["out"] + b_proj
    return out
